# revision 1
# baseline (speedup 1.0000x reference)
"""DetectionLoss Trainium2 Bass kernel.

Data-parallel over batch: 2 images per core x 8 cores; host sums 18 partial
sums per core (npos is a global normalizer, so per-core normalization is
impossible anyway - the sharding hint's "per-shard sums + counts").

Device algorithm per core:
  sparse path (starts immediately): box cells -> 128x128 same-cell masks
  (last-box-wins winners, min-label targets) -> indirect gathers of the
  per-cell records (obj, reg0..3) and cls logit at the target class ->
  smooth-L1 and CE numerators.
  dense path (overlapped): sum_k exp(cls[k, cell]) for every cell via
  bf16 matmul against a block-selector, staged to DRAM, gathered back at
  the <=128 positive cells for the logsumexp term; softplus over all obj
  logits via Exp+Ln(x+1) (gen3 ACT tables lack Softplus).

The obj+reg inputs are repacked on host into per-cell records [2HW, 5]
(pure relayout - all arithmetic happens on device) so one indirect DMA per
scale fetches all five values per box; indirect DMAs cost ~1.1us each on
GPSIMD and were the dominant serial chain in v1.
"""

import numpy as np
import ml_dtypes

import concourse.bass as bass
import concourse.tile as tile
from concourse import bacc, mybir
from concourse.bass_utils import run_bass_kernel_spmd
from concourse.tile_rust import add_dep_helper

F32 = mybir.dt.float32
BF16 = mybir.dt.bfloat16
I32 = mybir.dt.int32
AF = mybir.ActivationFunctionType
OP = mybir.AluOpType
AX = mybir.AxisListType

B_TOT = 16
N_CORES = 8
B_SH = B_TOT // N_CORES
NBOX = 64
NP = B_SH * NBOX  # 128 partitions: (image, box)
C = 30
SCALES = [(80, 80), (40, 40), (20, 20)]
BIG = 1.0e9
CHUNK = 400  # divides every HW/2; psum [4*nch, 400] fits one bank

CLS_W, REG_W, OBJ_W = 1.0, 5.0, 1.0
NPART = 18  # per scale s, cols 6s + [lse, clsval, sl1, obj, softplus, npos]


def _consts():
    ident = np.eye(128, dtype=np.float32)
    utri = np.triu(np.ones((128, 128), np.float32), 1)
    big = np.concatenate([ident, utri], axis=1)  # [128, 256]

    p = np.arange(128)
    bvec = (p >= NBOX).astype(np.float32)
    kc = np.zeros((128, 24), np.float32)
    for s, (h, w) in enumerate(SCALES):
        hw = h * w
        kc[:, 0 + s] = w          # W
        kc[:, 3 + s] = h          # H
        kc[:, 6 + s] = w - 1
        kc[:, 9 + s] = h - 1
        kc[:, 12 + s] = bvec * hw          # key offset
        kc[:, 15 + s] = bvec * C * hw      # cls gather offset
        kc[:, 18 + s] = hw                 # for minlab*HW

    # [120, 4]: partition (b, k, u) -> column (b*2 + u)
    bsel = np.zeros((120, 4), ml_dtypes.bfloat16)
    for pp in range(120):
        b = pp // 60
        u = pp % 2
        bsel[pp, b * 2 + u] = 1.0

    ones = np.ones((128, 1), np.float32)
    return big, kc, bsel, ones


def emit(tc: tile.TileContext, outs, ins):
    """outs: partials AP [18]; ins: dict name -> AP (per-core shard shapes)."""
    nc = tc.nc
    out_ap = outs

    big_c, kc_c, bsel_c, ones_c = _consts()
    big_h = nc.inline_tensor(big_c, name="cbig")
    kc_h = nc.inline_tensor(kc_c, name="ckc")
    bsel_h = nc.inline_tensor(bsel_c, name="cbsel")
    ones_h = nc.inline_tensor(ones_c, name="cones")

    pools = []

    def mkpool(**kw):
        p = tc.alloc_tile_pool(**kw)
        pools.append(p)
        return p

    pool = mkpool(name="sb", bufs=1)
    seps = mkpool(name="seps", bufs=3, space="PSUM")
    kmps = mkpool(name="kmps", bufs=2, space="PSUM")
    lbps = mkpool(name="lbps", bufs=1, space="PSUM")
    fips = mkpool(name="fips", bufs=1, space="PSUM")

    # ---- tiny inputs first: the sparse chain is the critical path ----
    btile = pool.tile([NP, 4], F32, tag="btile")
    nc.sync.dma_start(out=btile[:], in_=ins["boxes"].rearrange("b n c -> (b n) c"))
    kct = pool.tile([128, 24], F32, tag="kct")
    nc.sync.dma_start(out=kct[:], in_=kc_h.ap())
    labi = pool.tile([NP, 1], I32, tag="labi")
    nc.sync.dma_start(out=labi[:], in_=ins["labels"].rearrange("b n -> (b n)")[:, None])
    bigt = pool.tile([128, 256], F32, tag="bigt")
    nc.sync.dma_start(out=bigt[:], in_=big_h.ap())
    utri = bigt[:, 128:256]
    bselt = pool.tile([120, 4], BF16, tag="bselt")
    nc.sync.dma_start(out=bselt[:], in_=bsel_h.ap())

    # ---- batched (all scales) box -> cell/key indices ----
    # floor(x) = round-to-nearest(x - 0.5): HW f32->i32 convert rounds.
    # gxy [128, (coord, scale)] does x and y for all 3 scales per op.
    kxy = kct[:, 0:6].rearrange("p (c s) -> p c s", c=2)
    kxy_clip = kct[:, 6:12].rearrange("p (c s) -> p c s", c=2)
    gr = pool.tile([NP, 2, 3], F32, tag="gr")
    nc.vector.tensor_tensor(
        out=gr[:], in0=btile[:, 0:2, None].to_broadcast([NP, 2, 3]), in1=kxy, op=OP.mult
    )
    nc.vector.tensor_scalar(out=gr[:], in0=gr[:], scalar1=-0.5, scalar2=None, op0=OP.add)
    gi = pool.tile([NP, 2, 3], I32, tag="gi")
    nc.vector.tensor_copy(out=gi[:], in_=gr[:])
    gf = pool.tile([NP, 2, 3], F32, tag="gf")
    nc.vector.tensor_copy(out=gf[:], in_=gi[:])
    nc.vector.tensor_tensor(out=gf[:], in0=gf[:], in1=kxy_clip, op=OP.min)

    cellf = pool.tile([NP, 3], F32, tag="cellf")
    nc.vector.tensor_tensor(out=cellf[:], in0=gf[:, 1, :], in1=kct[:, 0:3], op=OP.mult)
    nc.vector.tensor_add(cellf[:], cellf[:], gf[:, 0, :])
    keyf = pool.tile([NP, 3], F32, tag="keyf")
    nc.vector.tensor_add(keyf[:], cellf[:], kct[:, 12:15])
    keyi = pool.tile([NP, 3], I32, tag="keyi")
    nc.vector.tensor_copy(out=keyi[:], in_=keyf[:])

    # ---- obj+reg record gathers: issue as soon as keys exist ----
    og_all = pool.tile([NP, 15], F32, tag="og_all")  # (obj, reg0..3) x 3 scales
    for s in range(3):
        nc.gpsimd.indirect_dma_start(
            out=og_all[:, 5 * s : 5 * s + 5],
            out_offset=None,
            in_=ins[f"objreg{s}"],
            in_offset=bass.IndirectOffsetOnAxis(ap=keyi[:, s : s + 1], axis=0),
        )

    labf = pool.tile([NP, 1], F32, tag="labf")
    nc.vector.tensor_copy(out=labf[:], in_=labi[:])
    stack = pool.tile([128, NPART], F32, tag="stack")
    nc.vector.memset(stack[:], 0.0)
    stv = stack[:].rearrange("p (s j) -> p s j", j=6)

    # ---- key/label row matrices: PE transpose of broadcast columns ----
    # (labmat[p, q] = labf[q]; keymat_s[p, q] = keyf[q, s])
    labmat = lbps.tile([128, 128], F32, tag="labmat")
    nc.tensor.transpose(
        out=labmat[:], in_=labf[:].to_broadcast([128, 128]), identity=bigt[:, 0:128]
    )

    # ---- per-scale masks: winners (last box wins) + min same-cell label ----
    win3 = pool.tile([NP, 3], F32, tag="win3")
    minlab3 = pool.tile([NP, 3], F32, tag="minlab3")
    for s in range(3):
        kmat = kmps.tile([128, 128], F32, tag="kmat")
        nc.tensor.transpose(
            out=kmat[:],
            in_=keyf[:, s : s + 1].to_broadcast([128, 128]),
            identity=bigt[:, 0:128],
        )
        eqm = pool.tile([128, 128], F32, tag=f"eqm{s}")
        nc.vector.tensor_scalar(
            out=eqm[:], in0=kmat[:], scalar1=keyf[:, s : s + 1], scalar2=None, op0=OP.is_equal
        )
        lose = pool.tile([128, 128], F32, tag=f"lose{s}")
        nc.vector.tensor_mul(lose[:], eqm[:], utri)
        losev = pool.tile([NP, 1], F32, tag=f"losev{s}")
        nc.vector.tensor_reduce(out=losev[:], in_=lose[:], axis=AX.X, op=OP.max)
        nc.vector.tensor_scalar(
            out=win3[:, s : s + 1], in0=losev[:], scalar1=-1.0, scalar2=1.0, op0=OP.mult, op1=OP.add
        )
        cnd = pool.tile([128, 128], F32, tag=f"cnd{s}")
        nc.vector.tensor_scalar(
            out=cnd[:], in0=eqm[:], scalar1=-BIG, scalar2=BIG, op0=OP.mult, op1=OP.add
        )
        nc.vector.tensor_tensor(out=cnd[:], in0=cnd[:], in1=labmat[:], op=OP.add)
        nc.vector.tensor_reduce(out=minlab3[:, s : s + 1], in_=cnd[:], axis=AX.X, op=OP.min)

    cidxf = pool.tile([NP, 3], F32, tag="cidxf")
    nc.vector.tensor_tensor(out=cidxf[:], in0=minlab3[:], in1=kct[:, 18:21], op=OP.mult)
    nc.vector.tensor_add(cidxf[:], cidxf[:], cellf[:])
    nc.vector.tensor_add(cidxf[:], cidxf[:], kct[:, 15:18])
    cidxi = pool.tile([NP, 3], I32, tag="cidxi")
    nc.vector.tensor_copy(out=cidxi[:], in_=cidxf[:])

    # ---- cls-logit-at-target-class gathers ----
    clsv3 = pool.tile([NP, 3], F32, tag="clsv3")
    for s in range(3):
        nc.gpsimd.indirect_dma_start(
            out=clsv3[:, s : s + 1],
            out_offset=None,
            in_=ins[f"cls_p{s}"].rearrange("b k h w -> (b k h w)")[:, None],
            in_offset=bass.IndirectOffsetOnAxis(ap=cidxi[:, s : s + 1], axis=0),
        )

    # ---- dense phase, smallest scale first so its se-gather issues early.
    # cls loads go on the scalar HWDGE queue (sync queue holds the small
    # early loads + se writes); all Exp ACT ops are emitted before any Ln
    # to avoid ping-ponging activation-table loads (1.28us each).
    se_h = [
        nc.dram_tensor(f"se{s}", (B_SH * h * w,), F32, kind="Internal")
        for s, (h, w) in enumerate(SCALES)
    ]
    seg3 = pool.tile([NP, 3], F32, tag="seg3")
    obj_ln = []
    se_wr = {}
    for s, (H, W) in enumerate(SCALES):
        HW = H * W
        HW2 = HW // 2
        nch = HW2 // CHUNK if HW2 >= CHUNK else 1
        csz = HW2 // nch  # 400, 400, 200
        cls_pf = ins[f"cls_p{s}"].rearrange("b k (u f) w -> (b k u) (f w)", u=2)

        expt = pool.tile([120, HW2], BF16, tag=f"expt{s}")
        ndma = 2 if s == 0 else 1
        dsz = HW2 // ndma
        for di in range(ndma):
            ct = pool.tile([120, dsz], F32, tag=f"clsin{s}_{di}")
            nc.scalar.dma_start(out=ct[:], in_=cls_pf[:, di * dsz : (di + 1) * dsz])
            nc.scalar.activation(out=expt[:, di * dsz : (di + 1) * dsz], in_=ct[:], func=AF.Exp)

        # obj softplus: exp now, ln later (batched with the other Lns)
        p_obj = 128 if s < 2 else 32
        n_rec = B_SH * HW // p_obj
        objt = pool.tile([p_obj, n_rec * 5], F32, tag=f"objt{s}")
        nc.sync.dma_start(
            out=objt[:], in_=ins[f"objreg{s}"].rearrange("v r -> (v r)").rearrange("(p f) -> p f", p=p_obj)
        )
        objv = objt[:].rearrange("p (j r) -> p j r", r=5)[:, :, 0]
        obje = pool.tile([p_obj, n_rec], F32, tag=f"obje{s}")
        nc.scalar.activation(out=obje[:], in_=objv, func=AF.Exp)
        obj_ln.append((s, p_obj, n_rec, obje))

        sesb = pool.tile([4, HW2], F32, tag=f"sesb{s}")
        for ci in range(nch):
            se_ps = seps.tile([4, csz], F32, tag="seps")
            nc.tensor.matmul(
                out=se_ps[:],
                lhsT=bselt[:],
                rhs=expt[:, ci * csz : (ci + 1) * csz],
                start=True,
                stop=True,
            )
            nc.vector.tensor_copy(out=sesb[:, ci * csz : (ci + 1) * csz], in_=se_ps[:])
        # se flat layout is (b, u, j) = row-major [4, HW2]
        se_wr[s] = nc.sync.dma_start(
            out=se_h[s].ap().rearrange("(p f) -> p f", p=4), in_=sesb[:]
        )

    # se gathers ordered by expected write-completion time (s0's dense
    # pipeline is gated by the big cls0 transfer and finishes last)
    for s in (1, 2, 0):
        g = nc.gpsimd.indirect_dma_start(
            out=seg3[:, s : s + 1],
            out_offset=None,
            in_=se_h[s].ap()[:, None],
            in_offset=bass.IndirectOffsetOnAxis(ap=keyi[:, s : s + 1], axis=0),
        )
        add_dep_helper(g.ins, se_wr[s].ins, reason="se scratch RAW")

    # ---- smooth-L1 over gathered reg records (emitted late: depends on
    # gather DATA, which lands ~3us after issue under bulk-DMA contention;
    # anything DVE emitted after this would head-of-line stall) ----
    ogv = og_all[:].rearrange("p (s r) -> p s r", r=5)
    d12 = pool.tile([NP, 3, 4], F32, tag="d12")
    nc.vector.tensor_tensor(
        out=d12[:], in0=ogv[:, :, 1:5], in1=btile[:, None, :].to_broadcast([NP, 3, 4]), op=OP.subtract
    )
    nc.scalar.activation(out=d12[:], in_=d12[:], func=AF.Abs)
    q12 = pool.tile([NP, 3, 4], F32, tag="q12")
    nc.vector.tensor_scalar_min(q12[:], d12[:], 1.0)
    h12 = pool.tile([NP, 3, 4], F32, tag="h12")
    nc.vector.tensor_scalar(out=h12[:], in0=q12[:], scalar1=-0.5, scalar2=None, op0=OP.mult)
    nc.vector.tensor_add(h12[:], h12[:], d12[:])
    nc.vector.tensor_mul(h12[:], h12[:], q12[:])
    sl13 = pool.tile([NP, 3], F32, tag="sl13")
    nc.vector.tensor_reduce(out=sl13[:], in_=h12[:], axis=AX.X, op=OP.add)
    nc.vector.tensor_scalar(out=sl13[:], in0=sl13[:], scalar1=0.25, scalar2=None, op0=OP.mult)
    nc.vector.tensor_scalar_min(sl13[:], sl13[:], 10.0)
    nc.vector.tensor_mul(stv[:, :, 1], clsv3[:], win3[:])
    nc.vector.tensor_mul(stv[:, :, 2], sl13[:], win3[:])
    nc.vector.tensor_mul(stv[:, :, 3], ogv[:, :, 0], win3[:])
    nc.vector.tensor_copy(out=stv[:, :, 5], in_=win3[:])

    for s, p_obj, n_rec, obje in obj_ln:
        objl = pool.tile([p_obj, n_rec], F32, tag=f"objl{s}")
        nc.scalar.activation(
            out=objl[:], in_=obje[:], func=AF.Ln, bias=1.0,
            accum_out=stack[:p_obj, 6 * s + 4 : 6 * s + 5],
        )

    lse3 = pool.tile([NP, 3], F32, tag="lse3")
    nc.scalar.activation(out=lse3[:], in_=seg3[:], func=AF.Ln)
    nc.vector.tensor_mul(stv[:, :, 0], lse3[:], win3[:])

    # ---- final: transpose stack then sum along free (the v1 stack@ones
    # matmul showed a pathological 12us slice) ----
    finT = fips.tile([NPART, 128], F32, tag="finT")
    nc.tensor.transpose(out=finT[:], in_=stack[:], identity=bigt[:, 0:128])
    fin_sb = pool.tile([NPART, 1], F32, tag="fin_sb")
    nc.vector.tensor_reduce(out=fin_sb[:], in_=finT[:], axis=AX.X, op=OP.add)
    nc.sync.dma_start(out=out_ap, in_=fin_sb[:])

    for p in reversed(pools):
        p.release()


# ---------------------------------------------------------------------------
# host side
# ---------------------------------------------------------------------------

_CACHE = {}


def _build():
    if "nc" in _CACHE:
        return _CACHE["nc"]
    nc = bacc.Bacc(
        "TRN2",
        target_bir_lowering=False,
        debug=False,
        enable_asserts=False,
        num_devices=N_CORES,
    )
    ins = {}
    for s, (h, w) in enumerate(SCALES):
        ins[f"cls_p{s}"] = nc.dram_tensor(f"cls_p{s}", (B_SH, C, h, w), F32, kind="ExternalInput").ap()
        ins[f"objreg{s}"] = nc.dram_tensor(f"objreg{s}", (B_SH * h * w, 5), F32, kind="ExternalInput").ap()
    ins["boxes"] = nc.dram_tensor("boxes", (B_SH, NBOX, 4), F32, kind="ExternalInput").ap()
    ins["labels"] = nc.dram_tensor("labels", (B_SH, NBOX), I32, kind="ExternalInput").ap()
    out = nc.dram_tensor("partials", (NPART,), F32, kind="ExternalOutput").ap()

    with tile.TileContext(nc) as tc:
        emit(tc, out, ins)
    nc.compile()
    _CACHE["nc"] = nc
    return nc


def make_objreg(obj_slice, reg_slice):
    """[b,1,H,W] obj + [b,4,H,W] reg -> per-cell records [b*H*W, 5]."""
    b = obj_slice.shape[0]
    hw = obj_slice.shape[2] * obj_slice.shape[3]
    rec = np.empty((b * hw, 5), np.float32)
    rec[:, 0] = np.asarray(obj_slice).reshape(-1)
    rec[:, 1:] = np.asarray(reg_slice).reshape(b, 4, hw).transpose(0, 2, 1).reshape(b * hw, 4)
    return rec


def combine_partials(parts):
    """parts: [n_cores, 18] -> final [4] losses."""
    tot = np.asarray(parts, np.float64).sum(axis=0)
    cls_sum = reg_sum = obj_sum = 0.0
    for s, (h, w) in enumerate(SCALES):
        b = 6 * s
        lse, val, sl1, obj, sp, npos = tot[b : b + 6]
        npos = max(npos, 1.0)
        cls_sum += (lse - val) / npos * CLS_W
        reg_sum += sl1 / npos * REG_W
        obj_sum += (sp - obj) / (B_TOT * h * w) * OBJ_W
    cls_sum /= len(SCALES)
    reg_sum /= len(SCALES)
    obj_sum /= len(SCALES)
    total = cls_sum + reg_sum + obj_sum
    return np.array([total, cls_sum, reg_sum, obj_sum], np.float32)


TRACE = False
LAST_RESULT = None


def kernel(**inputs):
    global LAST_RESULT
    nc = _build()
    in_maps = []
    for c in range(N_CORES):
        lo, hi = c * B_SH, (c + 1) * B_SH
        m = {}
        for s in range(3):
            m[f"cls_p{s}"] = np.ascontiguousarray(inputs[f"cls_p{s}"][lo:hi])
            m[f"objreg{s}"] = make_objreg(
                inputs[f"obj_p{s}"][lo:hi], inputs[f"reg_p{s}"][lo:hi]
            )
        m["boxes"] = np.ascontiguousarray(inputs["boxes"][lo:hi])
        m["labels"] = np.ascontiguousarray(inputs["labels"][lo:hi])
        in_maps.append(m)
    res = run_bass_kernel_spmd(
        nc, in_maps, core_ids=list(range(N_CORES)), trace=TRACE
    )
    LAST_RESULT = res
    parts = np.stack([np.asarray(r["partials"]) for r in res.results])
    return combine_partials(parts)



# revision 2
# speedup vs baseline: 1.5418x; 1.5418x over previous
"""DetectionLoss Trainium2 Bass kernel (v2: sparse-only).

Data-parallel over batch: 2 images per core x 8 cores; host sums 18 partial
sums per core (npos is a global normalizer, so per-core normalization is
impossible anyway - the sharding hint's "per-shard sums + counts").

v1 (43.4us) computed the CE denominator densely: exp over all 384k cls
logits, bf16 matmul-reduce to per-cell sum-exp, a DRAM round trip, and an
indirect gather back at the <=128 positive cells. But the reference only
NEEDS logsumexp at positive cells, so v2 repacks (host-side, pure relayout)
obj/reg/cls into per-cell records [B*HW, 64] (obj, reg0..3, cls0..29, pad
to a 256B row) and fetches the <=384 needed rows with three indirect DMAs
(one per scale, ~1.4us each on GpSimd - measured; a single dma_gather was
probed at ~11us end-to-end and rejected). The whole dense phase collapses
to the obj softplus over 16.8k logits, padded with -88 so pad cells add
exactly 0.

Device algorithm per core:
  keys (DVE): box -> per-scale record row = base_s + b*HW_s + gy*W+gx.
  gathers (GpSimd): 3 indirect DMAs -> rec [128, 3, 64].
  masks (PE+DVE, overlapped with gathers): 128x128 same-cell compare ->
  last-box-wins winners + min same-cell label (torch argmax tie rule).
  CE (ACT+DVE): exp of gathered 30-logit rows, row-sum, ln; target logit
  picked with an iota==minlab one-hot. Exp ops all emitted before any Ln
  (one act-table swap, 1.28us, instead of three).
  smooth-L1 (DVE) on gathered reg vs box; obj softplus dense via
  Exp+Ln(x+1) with per-scale accum.
  18 partial sums -> PE transpose -> DVE reduce -> DMA out.
"""

import numpy as np

import concourse.bass as bass
import concourse.tile as tile
from concourse import bacc, mybir
from concourse.bass_utils import run_bass_kernel_spmd

F32 = mybir.dt.float32
I32 = mybir.dt.int32
AF = mybir.ActivationFunctionType
OP = mybir.AluOpType
AX = mybir.AxisListType

B_TOT = 16
N_CORES = 8
B_SH = B_TOT // N_CORES
NBOX = 64
NP = B_SH * NBOX  # 128 partitions: (image, box)
C = 30
SCALES = [(80, 80), (40, 40), (20, 20)]
HWS = [h * w for h, w in SCALES]
REC_BASE = [0, B_SH * HWS[0], B_SH * (HWS[0] + HWS[1])]  # 0, 12800, 16000
NREC = B_SH * sum(HWS)  # 16800
RECW = 64  # padded record row (256B, also dma_gather-compatible)
BIG = 1.0e9
OBJ_COLS = [100, 25, 7]  # 12800=128x100, 3200=128x25, 800 -> 128x7 padded
OBJ_PAD = -88.0  # softplus(-88) == 0 exactly in f32

CLS_W, REG_W, OBJ_W = 1.0, 5.0, 1.0
NPART = 18  # per scale s, cols 6s + [lse, clsval, sl1, obj, softplus, npos]


def _consts():
    ident = np.eye(128, dtype=np.float32)
    utri = np.triu(np.ones((128, 128), np.float32), 1)
    big = np.concatenate([ident, utri], axis=1)  # [128, 256]

    p = np.arange(128)
    bvec = (p >= NBOX).astype(np.float32)
    kc = np.zeros((128, 15), np.float32)
    for s, (h, w) in enumerate(SCALES):
        kc[:, 0 + s] = w          # x multiplier
        kc[:, 3 + s] = h          # y multiplier
        kc[:, 6 + s] = w - 1      # x clip
        kc[:, 9 + s] = h - 1      # y clip
        kc[:, 12 + s] = REC_BASE[s] + bvec * (h * w)  # record-row offset

    iota = np.tile(np.arange(C, dtype=np.float32), (128, 1))  # [128, 30]
    return big, kc, iota


def emit(tc: tile.TileContext, out_ap, ins):
    nc = tc.nc

    big_c, kc_c, iota_c = _consts()
    big_h = nc.inline_tensor(big_c, name="cbig")
    kc_h = nc.inline_tensor(kc_c, name="ckc")
    iota_h = nc.inline_tensor(iota_c, name="ciota")

    pool = tc.alloc_tile_pool(name="sb", bufs=1)
    kmps = tc.alloc_tile_pool(name="kmps", bufs=1, space="PSUM")
    lbps = tc.alloc_tile_pool(name="lbps", bufs=1, space="PSUM")
    fips = tc.alloc_tile_pool(name="fips", bufs=1, space="PSUM")

    # ---- input DMAs: boxes first (keys are the critical path) ----
    btile = pool.tile([NP, 4], F32, tag="btile")
    nc.sync.dma_start(out=btile[:], in_=ins["boxes"].rearrange("b n c -> (b n) c"))
    kct = pool.tile([128, 15], F32, tag="kct")
    nc.sync.dma_start(out=kct[:], in_=kc_h.ap())
    labi = pool.tile([NP, 1], I32, tag="labi")
    nc.sync.dma_start(out=labi[:], in_=ins["labels"].rearrange("b n -> (b n)")[:, None])
    bigt = pool.tile([128, 256], F32, tag="bigt")
    nc.sync.dma_start(out=bigt[:], in_=big_h.ap())
    iott = pool.tile([128, C], F32, tag="iott")
    nc.sync.dma_start(out=iott[:], in_=iota_h.ap())
    utri = bigt[:, 128:256]

    objd = pool.tile([128, sum(OBJ_COLS)], F32, tag="objd")
    nc.scalar.dma_start(out=objd[:], in_=ins["objd"])

    # prefetch the Exp act table (1.28us) while inputs stream in
    dmy = pool.tile([128, 1], F32, tag="dmy")
    nc.scalar.activation(out=dmy[:], in_=kct[:, 0:1], func=AF.Exp, scale=0.0)

    # ---- batched (all scales) box -> record-row keys ----
    # floor(x) = round-to-nearest(x - 0.5): HW f32->i32 convert rounds.
    kxy = kct[:, 0:6].rearrange("p (c s) -> p c s", c=2)
    kclip = kct[:, 6:12].rearrange("p (c s) -> p c s", c=2)
    gr = pool.tile([NP, 2, 3], F32, tag="gr")
    nc.vector.tensor_tensor(
        out=gr[:], in0=btile[:, 0:2, None].to_broadcast([NP, 2, 3]), in1=kxy, op=OP.mult
    )
    nc.vector.tensor_scalar(out=gr[:], in0=gr[:], scalar1=-0.5, scalar2=None, op0=OP.add)
    gi = pool.tile([NP, 2, 3], I32, tag="gi")
    nc.vector.tensor_copy(out=gi[:], in_=gr[:])
    gf = pool.tile([NP, 2, 3], F32, tag="gf")
    nc.vector.tensor_copy(out=gf[:], in_=gi[:])
    nc.vector.tensor_tensor(out=gf[:], in0=gf[:], in1=kclip, op=OP.min)

    keyf = pool.tile([NP, 3], F32, tag="keyf")
    nc.vector.tensor_tensor(out=keyf[:], in0=gf[:, 1, :], in1=kct[:, 0:3], op=OP.mult)
    nc.vector.tensor_add(keyf[:], keyf[:], gf[:, 0, :])
    nc.vector.tensor_add(keyf[:], keyf[:], kct[:, 12:15])
    keyi = pool.tile([NP, 3], I32, tag="keyi")
    nc.vector.tensor_copy(out=keyi[:], in_=keyf[:])

    # ---- record gathers: one indirect DMA per scale, ~1.4us each ----
    rec = pool.tile([NP, 3 * RECW], F32, tag="rec")
    recv = rec[:].rearrange("p (s r) -> p s r", r=RECW)
    for s in range(3):
        nc.gpsimd.indirect_dma_start(
            out=recv[:, s, :],
            out_offset=None,
            in_=ins["rec"],
            in_offset=bass.IndirectOffsetOnAxis(ap=keyi[:, s : s + 1], axis=0),
        )

    # ---- masks (PE+DVE), fully overlapped with the gather flight ----
    labf = pool.tile([NP, 1], F32, tag="labf")
    nc.vector.tensor_copy(out=labf[:], in_=labi[:])
    labmat = lbps.tile([128, 128], F32, tag="labmat")
    nc.tensor.transpose(
        out=labmat[:], in_=labf[:].to_broadcast([128, 128]), identity=bigt[:, 0:128]
    )
    kmat = kmps.tile([128, 3 * 128], F32, tag="kmat")
    kmv = kmat[:].rearrange("p (s q) -> p s q", q=128)
    for s in range(3):
        nc.tensor.transpose(
            out=kmv[:, s, :],
            in_=keyf[:, s : s + 1].to_broadcast([128, 128]),
            identity=bigt[:, 0:128],
        )

    eqm = pool.tile([128, 3, 128], F32, tag="eqm")
    nc.vector.tensor_tensor(
        out=eqm[:], in0=kmv, in1=keyf[:, :, None].to_broadcast([128, 3, 128]),
        op=OP.is_equal,
    )
    lose = pool.tile([128, 3, 128], F32, tag="lose")
    nc.vector.tensor_tensor(
        out=lose[:], in0=eqm[:], in1=utri[:, None, :].to_broadcast([128, 3, 128]),
        op=OP.mult,
    )
    win3 = pool.tile([NP, 3], F32, tag="win3")
    nc.vector.tensor_reduce(out=win3[:], in_=lose[:], axis=AX.X, op=OP.max)
    nc.vector.tensor_scalar(
        out=win3[:], in0=win3[:], scalar1=-1.0, scalar2=1.0, op0=OP.mult, op1=OP.add
    )
    cnd = pool.tile([128, 3, 128], F32, tag="cnd")
    nc.vector.tensor_scalar(
        out=cnd[:], in0=eqm[:], scalar1=-BIG, scalar2=BIG, op0=OP.mult, op1=OP.add
    )
    nc.vector.tensor_tensor(
        out=cnd[:], in0=cnd[:], in1=labmat[:, None, :].to_broadcast([128, 3, 128]),
        op=OP.add,
    )
    minlab3 = pool.tile([NP, 3], F32, tag="minlab3")
    nc.vector.tensor_reduce(out=minlab3[:], in_=cnd[:], axis=AX.X, op=OP.min)

    # ---- scalar chain: obj exp early; CE exp after gather; then all Lns ----
    obje = pool.tile([128, sum(OBJ_COLS)], F32, tag="obje")
    nc.scalar.activation(out=obje[:], in_=objd[:], func=AF.Exp)

    expc = pool.tile([NP, 3, C], F32, tag="expc")
    nc.scalar.activation(out=expc[:], in_=recv[:, :, 5 : 5 + C], func=AF.Exp)

    stack = pool.tile([128, NPART], F32, tag="stack")
    stv = stack[:].rearrange("p (s j) -> p s j", j=6)

    se3 = pool.tile([NP, 3], F32, tag="se3")
    nc.vector.tensor_reduce(out=se3[:], in_=expc[:], axis=AX.X, op=OP.add)
    lse3 = pool.tile([NP, 3], F32, tag="lse3")
    nc.scalar.activation(out=lse3[:], in_=se3[:], func=AF.Ln)

    c0 = 0
    for s in range(3):
        objl = pool.tile([128, OBJ_COLS[s]], F32, tag=f"objl{s}")
        nc.scalar.activation(
            out=objl[:], in_=obje[:, c0 : c0 + OBJ_COLS[s]], func=AF.Ln, bias=1.0,
            accum_out=stack[:, 6 * s + 4 : 6 * s + 5],
        )
        c0 += OBJ_COLS[s]

    # ---- smooth-L1 over gathered reg records (DVE, after masks) ----
    d3 = pool.tile([NP, 3, 4], F32, tag="d3")
    nc.vector.tensor_tensor(
        out=d3[:], in0=recv[:, :, 1:5], in1=btile[:, None, :].to_broadcast([NP, 3, 4]),
        op=OP.subtract,
    )
    a3 = pool.tile([NP, 3, 4], F32, tag="a3")
    nc.vector.tensor_scalar(out=a3[:], in0=d3[:], scalar1=-1.0, scalar2=None, op0=OP.mult)
    nc.vector.tensor_tensor(out=a3[:], in0=a3[:], in1=d3[:], op=OP.max)
    q3 = pool.tile([NP, 3, 4], F32, tag="q3")
    nc.vector.tensor_scalar_min(q3[:], a3[:], 1.0)
    h3 = pool.tile([NP, 3, 4], F32, tag="h3")
    nc.vector.tensor_scalar(out=h3[:], in0=q3[:], scalar1=-0.5, scalar2=None, op0=OP.mult)
    nc.vector.tensor_add(h3[:], h3[:], a3[:])
    nc.vector.tensor_mul(h3[:], h3[:], q3[:])
    sl13 = pool.tile([NP, 3], F32, tag="sl13")
    nc.vector.tensor_reduce(out=sl13[:], in_=h3[:], axis=AX.X, op=OP.add)
    nc.vector.tensor_scalar(out=sl13[:], in0=sl13[:], scalar1=0.25, scalar2=None, op0=OP.mult)
    nc.vector.tensor_scalar_min(sl13[:], sl13[:], 10.0)

    # ---- cls target logit: one-hot(minlab) dot gathered row ----
    eqc = pool.tile([NP, 3, C], F32, tag="eqc")
    nc.vector.tensor_tensor(
        out=eqc[:], in0=iott[:, None, :].to_broadcast([NP, 3, C]),
        in1=minlab3[:, :, None].to_broadcast([NP, 3, C]), op=OP.is_equal,
    )
    nc.vector.tensor_mul(eqc[:], eqc[:], recv[:, :, 5 : 5 + C])
    val3 = pool.tile([NP, 3], F32, tag="val3")
    nc.vector.tensor_reduce(out=val3[:], in_=eqc[:], axis=AX.X, op=OP.add)

    # ---- stack the masked partials ----
    nc.vector.tensor_mul(stv[:, :, 1], val3[:], win3[:])
    nc.vector.tensor_mul(stv[:, :, 2], sl13[:], win3[:])
    nc.vector.tensor_mul(stv[:, :, 3], recv[:, :, 0], win3[:])
    nc.vector.tensor_copy(out=stv[:, :, 5], in_=win3[:])
    nc.vector.tensor_mul(stv[:, :, 0], lse3[:], win3[:])

    # ---- final: transpose stack, sum along free, DMA out ----
    finT = fips.tile([NPART, 128], F32, tag="finT")
    nc.tensor.transpose(out=finT[:], in_=stack[:], identity=bigt[:, 0:128])
    fin_sb = pool.tile([NPART, 1], F32, tag="fin_sb")
    nc.vector.tensor_reduce(out=fin_sb[:], in_=finT[:], axis=AX.X, op=OP.add)
    nc.sync.dma_start(out=out_ap, in_=fin_sb[:])

    fips.release()
    lbps.release()
    kmps.release()
    pool.release()


# ---------------------------------------------------------------------------
# host side
# ---------------------------------------------------------------------------

_CACHE = {}


def _build():
    if "nc" in _CACHE:
        return _CACHE["nc"]
    nc = bacc.Bacc(
        "TRN2",
        target_bir_lowering=False,
        debug=False,
        enable_asserts=False,
        num_devices=N_CORES,
    )
    ins = {
        "rec": nc.dram_tensor("rec", (NREC, RECW), F32, kind="ExternalInput").ap(),
        "objd": nc.dram_tensor("objd", (128, sum(OBJ_COLS)), F32, kind="ExternalInput").ap(),
        "boxes": nc.dram_tensor("boxes", (B_SH, NBOX, 4), F32, kind="ExternalInput").ap(),
        "labels": nc.dram_tensor("labels", (B_SH, NBOX), I32, kind="ExternalInput").ap(),
    }
    out = nc.dram_tensor("partials", (NPART,), F32, kind="ExternalOutput").ap()

    with tile.TileContext(nc) as tc:
        emit(tc, out, ins)
    nc.compile()
    _CACHE["nc"] = nc
    return nc


def make_rec(inputs, lo, hi):
    """Per-cell records [16800, 64]: (obj, reg0..3, cls0..29, 0-pad).

    Pure relayout - all arithmetic happens on device. Row of cell (s,b,y,x)
    is REC_BASE[s] + b*HW_s + y*W_s + x.
    """
    rec = np.zeros((NREC, RECW), np.float32)
    for s, (h, w) in enumerate(SCALES):
        hw = h * w
        r0 = REC_BASE[s]
        n = B_SH * hw
        rec[r0 : r0 + n, 0] = np.asarray(inputs[f"obj_p{s}"][lo:hi]).reshape(n)
        rec[r0 : r0 + n, 1:5] = (
            np.asarray(inputs[f"reg_p{s}"][lo:hi])
            .reshape(B_SH, 4, hw).transpose(0, 2, 1).reshape(n, 4)
        )
        rec[r0 : r0 + n, 5 : 5 + C] = (
            np.asarray(inputs[f"cls_p{s}"][lo:hi])
            .reshape(B_SH, C, hw).transpose(0, 2, 1).reshape(n, C)
        )
    return rec


def make_objd(inputs, lo, hi):
    """All obj logits as one [128, 132] tile; pad = -88 (softplus == 0)."""
    od = np.empty((128, sum(OBJ_COLS)), np.float32)
    c0 = 0
    for s, ncol in enumerate(OBJ_COLS):
        flat = np.full(128 * ncol, OBJ_PAD, np.float32)
        v = np.asarray(inputs[f"obj_p{s}"][lo:hi]).reshape(-1)
        flat[: v.size] = v
        od[:, c0 : c0 + ncol] = flat.reshape(128, ncol)
        c0 += ncol
    return od


def combine_partials(parts):
    """parts: [n_cores, 18] -> final [4] losses."""
    tot = np.asarray(parts, np.float64).sum(axis=0)
    cls_sum = reg_sum = obj_sum = 0.0
    for s, (h, w) in enumerate(SCALES):
        b = 6 * s
        lse, val, sl1, obj, sp, npos = tot[b : b + 6]
        npos = max(npos, 1.0)
        cls_sum += (lse - val) / npos * CLS_W
        reg_sum += sl1 / npos * REG_W
        obj_sum += (sp - obj) / (B_TOT * h * w) * OBJ_W
    cls_sum /= len(SCALES)
    reg_sum /= len(SCALES)
    obj_sum /= len(SCALES)
    total = cls_sum + reg_sum + obj_sum
    return np.array([total, cls_sum, reg_sum, obj_sum], np.float32)


TRACE = False
LAST_RESULT = None


def kernel(**inputs):
    global LAST_RESULT
    nc = _build()
    in_maps = []
    for c in range(N_CORES):
        lo, hi = c * B_SH, (c + 1) * B_SH
        in_maps.append({
            "rec": make_rec(inputs, lo, hi),
            "objd": make_objd(inputs, lo, hi),
            "boxes": np.ascontiguousarray(inputs["boxes"][lo:hi]),
            "labels": np.ascontiguousarray(inputs["labels"][lo:hi]),
        })
    res = run_bass_kernel_spmd(
        nc, in_maps, core_ids=list(range(N_CORES)), trace=TRACE
    )
    LAST_RESULT = res
    parts = np.stack([np.asarray(r["partials"]) for r in res.results])
    return combine_partials(parts)
